# revision 1
# baseline (speedup 1.0000x reference)
"""Trainium2 Bass kernel for the dense_cnn problem (fp16/fp8 version).

Math (per sample, C=256, H=W=56, HW=3136, G=2, K=128):
  t1 = p1*x
  t2 = t1[c,hw] @ p2[hw,k]                  (computed transposed: t2T[k,c])
  t3 = t1 @ x.T / sqrt(hw)                  (computed transposed: t3T[d,c])
  t4 = grouped dilated 3x1 conv of t2 (only middle kw tap contributes)
  t5 = w5 @ x
  t8 = grouped dilated 3x3 conv of x (block-diag shifted matmuls)
  t9 = max(t5, t8)
  out = (t4 - t3/sqrt(hw)) @ t9 / sqrt(c)

Precision plan (validated vs reference: rel err ~1.3e-2 < 2e-2 gate):
  - fp16 for x shipping, phase-1 matmuls, t5, t7, t9, out (err ~6e-4)
  - fp8e4m3 for t8 only, via DoubleRow matmuls that pack two taps per
    pass (2x PE rate); t8's error is gated by max(t5,t8).
  - 1/sqrt(c) folded into t7 (w4 and the t3 STT scalar are pre-scaled).

Distribution: pure data-parallel over batch, 4 samples per core x 8 cores.
"""

import dataclasses

import numpy as np
import ml_dtypes

import concourse.bass as bass
import concourse.tile as tile
from concourse import bacc, mybir
from concourse.bass_utils import run_bass_kernel_spmd

N, C, H, W = 32, 256, 56, 56
HW = H * W              # 3136
PW = W + 6              # width padded by 3 each side: 62
HWP = H * PW            # 3472
NCORE = 8
SPC = N // NCORE        # samples per core: 4
NCHUNK = 25             # hw-contraction chunks of 128 (rows padded to 3200)
HWPAD = NCHUNK * 128    # 3200
NCH2 = 7                # phase-2 column chunks (8 image rows each)
CH_SP = HWP // NCH2     # 496 padded cols per chunk
CH_OUT = 448            # compact cols per chunk
XM = 192                # zero margin around each padded half (> max |shift| 189)
HWPM = HWP + 2 * XM     # 3856

F32 = mybir.dt.float32
F16 = mybir.dt.float16
F8 = mybir.dt.float8e4
DR = mybir.MatmulPerfMode.DoubleRow
MUL = mybir.AluOpType.mult
ADD = mybir.AluOpType.add

# t8 tap pairs: (kh,kw) shifts sh = 186*(kh-1) + 3*(kw-1); last pair solo
TAPS = [(kh, kw) for kh in range(3) for kw in range(3)]
TAP_PAIRS = [(TAPS[0], TAPS[1]), (TAPS[2], TAPS[3]), (TAPS[4], TAPS[5]),
             (TAPS[6], TAPS[7]), (TAPS[8], None)]


def _sh(tap):
    kh, kw = tap
    return 186 * (kh - 1) + 3 * (kw - 1)


_PROGRAM_CACHE: dict = {}


def _build_program():
    nc = bacc.Bacc("TRN2", target_bir_lowering=False, debug=False,
                   num_devices=NCORE)

    d = {}
    def din(name, shape, dt=F16):
        d[name] = nc.dram_tensor(name, list(shape), dt, kind="ExternalInput").ap()
    din("xt", (SPC, NCHUNK, 128, 256))
    din("xpad8", (SPC, 2, 128, HWPM), F8)
    din("xpad8l", (SPC, 2, 128, HWPM), F8)
    din("p1t", (NCHUNK, 128, 256))
    din("p2f", (NCHUNK, 128, 128))
    din("w4t", (3, 2, 128, 128))
    din("w8q", (2, 5, 128, 2, 128), F8)
    din("w5t8", (2, 3, 128, 2, 128), F8)
    din("ident", (128, 128))
    out_dram = nc.dram_tensor("out", [SPC, 2, 128, HW], F16,
                              kind="ExternalOutput").ap()

    with tile.TileContext(nc) as tc:
        _emit(tc, nc, d, out_dram)
    nc.compile()
    return nc


def _mov3(ap, off, d1, n1, d2, n2):
    """Strided (possibly overlapping) 3D view [128, n1, n2] of a 2D tile."""
    return dataclasses.replace(
        ap, offset=ap.offset + off, ap=[ap.ap[0], [d1, n1], [d2, n2]])


def _emit_out(nc, small_ps, t7T_sb, t9_sb, o_stage, j):
    """Final out-matmuls and staging copy for column chunk j."""
    for ct in range(2):
        o_ps = small_ps.tile([128, CH_OUT], F32, name="o_ps", tag="smps")
        for g in range(2):
            nc.tensor.matmul(
                o_ps[:], t7T_sb[g][:, ct * 128:(ct + 1) * 128], t9_sb[g][:],
                start=(g == 0), stop=(g == 1))
        nc.scalar.copy(
            o_stage[:, ct * HW + j * CH_OUT:ct * HW + (j + 1) * CH_OUT],
            o_ps[:])


def _emit(tc, nc, d, out_dram):
    from contextlib import ExitStack
    ctx = ExitStack()
    with ctx:
        const = ctx.enter_context(tc.tile_pool(name="const", bufs=1))
        xt_pool = ctx.enter_context(tc.tile_pool(name="xt", bufs=4))
        t1_pool = ctx.enter_context(tc.tile_pool(name="t1", bufs=4))
        xp8_pool = ctx.enter_context(tc.tile_pool(name="xp8", bufs=2))
        t9_pool = ctx.enter_context(tc.tile_pool(name="t9", bufs=6))
        sb_small = ctx.enter_context(tc.tile_pool(name="sbs", bufs=2))
        out_pool = ctx.enter_context(tc.tile_pool(name="outp", bufs=3))
        # PSUM budget (8 banks): acc 2 + small(shared t4-chain/out) 2 +
        # t8 2 + t5 2 = 8
        acc_ps = ctx.enter_context(tc.tile_pool(name="accps", bufs=1, space="PSUM"))
        small_ps = ctx.enter_context(tc.tile_pool(name="smps", bufs=2, space="PSUM"))
        t8_psp = ctx.enter_context(tc.tile_pool(name="t8ps", bufs=2, space="PSUM"))
        t5_psp = ctx.enter_context(tc.tile_pool(name="t5ps", bufs=2, space="PSUM"))

        # ---- load constants (split so sample 0 can start fast) --------------
        HEAD = 5        # chunk-block size for interleaved startup DMAs
        p1t_sb = const.tile([128, NCHUNK * 256], F16)
        p1v = p1t_sb[:].rearrange("p (i f) -> p i f", i=NCHUNK)
        p1d = d["p1t"].rearrange("i p f -> p i f")
        p2f_sb = const.tile([128, NCHUNK * 128], F16)
        p2v = p2f_sb[:].rearrange("p (i f) -> p i f", i=NCHUNK)
        p2d = d["p2f"].rearrange("i p f -> p i f")
        w4t_sb = const.tile([128, 6 * 128], F16)
        w8q_sb = const.tile([128, 2 * 5 * 2 * 128], F8)
        w5t_sb = const.tile([128, 2 * 3 * 2 * 128], F8)
        id_sb = const.tile([128, 128], F16)
        def load_weights():
            nc.sync.dma_start(
                w4t_sb[:].rearrange("p (i f) -> p i f", i=6),
                d["w4t"].rearrange("a b p f -> p (a b) f"))
            nc.sync.dma_start(
                w8q_sb[:].rearrange("p (g i f) -> p (g i) f", g=2, i=5),
                d["w8q"].rearrange("g i p a f -> p (g i) (a f)"))
            nc.sync.dma_start(
                w5t_sb[:].rearrange("p (g i f) -> p (g i) f", g=2, i=3),
                d["w5t8"].rearrange("g i p a f -> p (g i) (a f)"))
            nc.sync.dma_start(id_sb[:], d["ident"])
        # padded t2 staging ([128, 2 x 134], pad cols stay zero)
        t2p_sb = const.tile([128, 2 * 134], F16)
        nc.gpsimd.memset(t2p_sb[:], 0.0)

        inv56 = float(1.0 / np.float32(np.sqrt(np.float32(HW))))
        inv16 = float(1.0 / np.float32(np.sqrt(np.float32(C))))
        stt_scale = -inv56 * inv16      # t7 = (w4/16-conv) - t3/(56*16)

        for s in range(SPC):
            # ---- sample DMAs up front -------------------------------------
            xt_t = xt_pool.tile([128, NCHUNK * 256], F16)
            xtv = xt_t[:].rearrange("p (i f) -> p i f", i=NCHUNK)
            xtd = d["xt"][s].rearrange("i p f -> p i f")
            if s == 0:
                # interleave p1/p2/xt chunk-blocks so phase 1 streams
                for b0 in range(0, NCHUNK, HEAD):
                    b1 = min(b0 + HEAD, NCHUNK)
                    nc.sync.dma_start(p1v[:, b0:b1], p1d[:, b0:b1])
                    nc.sync.dma_start(p2v[:, b0:b1], p2d[:, b0:b1])
                    nc.sync.dma_start(xtv[:, b0:b1], xtd[:, b0:b1])
            else:
                nc.sync.dma_start(xtv[:], xtd)
            if s == 0:
                load_weights()      # small; needed by the first t8 matmul
            xp8_t = xp8_pool.tile([128, 4 * HWPM], F8)
            for half in range(2):
                nc.sync.dma_start(
                    xp8_t[:, half * HWPM:(half + 1) * HWPM],
                    d["xpad8"][s, half])
                nc.sync.dma_start(
                    xp8_t[:, (2 + half) * HWPM:(3 + half) * HWPM],
                    d["xpad8l"][s, half])

            # ---- phase 1: hw-contraction accumulations ----------------------
            t2T_ps = acc_ps.tile([128, 256], F32, tag="t2T")
            t3T_ps = acc_ps.tile([128, 512], F32, tag="t3T")
            for i in range(NCHUNK):
                t1_t = t1_pool.tile([128, 256], F16)
                nc.vector.tensor_mul(t1_t[:], xtv[:, i],
                                     p1t_sb[:, i * 256:(i + 1) * 256])
                nc.tensor.matmul(t2T_ps[:], p2f_sb[:, i * 128:(i + 1) * 128],
                                 t1_t[:], start=(i == 0),
                                 stop=(i == NCHUNK - 1))
                # t3T halves share one PSUM bank (one zero region): only the
                # first matmul starts the group, only the last one stops it.
                for g in range(2):
                    nc.tensor.matmul(t3T_ps[:, g * 256:(g + 1) * 256],
                                     xt_t[:, i * 256 + g * 128:
                                          i * 256 + (g + 1) * 128],
                                     t1_t[:],
                                     start=(i == 0 and g == 0),
                                     stop=(i == NCHUNK - 1 and g == 1))

            # ---- phase 1b: t4 chain (tiny, fp16) ----------------------------
            t2T_sb = sb_small.tile([128, 256], F16, tag="t2Tsb")
            nc.scalar.copy(t2T_sb[:], t2T_ps[:])
            for t in range(2):
                t2_ps = small_ps.tile([128, 128], F16, tag="smps")
                nc.tensor.transpose(t2_ps[:], t2T_sb[:, t * 128:(t + 1) * 128],
                                    id_sb[:])
                nc.vector.tensor_copy(t2p_sb[:, t * 134 + 3:t * 134 + 131],
                                      t2_ps[:])
            t4T_sb = sb_small.tile([128, 256], F16, tag="t4Tsb")
            for t in range(2):
                t4_ps = small_ps.tile([128, 128], F32, tag="smps")
                for ki, kh in enumerate(range(3)):
                    nc.tensor.matmul(
                        t4_ps[:], w4t_sb[:, (kh * 2 + t) * 128:(kh * 2 + t + 1) * 128],
                        t2p_sb[:, t * 134 + 3 * kh:t * 134 + 3 * kh + 128],
                        start=(ki == 0), stop=(ki == 2))
                t4_sb = sb_small.tile([128, 128], F16, tag="t4sb")
                nc.scalar.copy(t4_sb[:], t4_ps[:])
                t4T_ps = small_ps.tile([128, 128], F16, tag="smps")
                nc.tensor.transpose(t4T_ps[:], t4_sb[:], id_sb[:])
                nc.vector.tensor_copy(t4T_sb[:, t * 128:(t + 1) * 128], t4T_ps[:])
            # t7T[g] = t4T/16 - t3T[g]/(56*16)   (w4t pre-scaled by 1/16)
            t7T_sb = [sb_small.tile([128, 256], F16, name=f"t7T{g}", tag=f"t7T{g}")
                      for g in range(2)]
            for g in range(2):
                nc.vector.scalar_tensor_tensor(
                    t7T_sb[g][:], t3T_ps[:, g * 256:(g + 1) * 256], stt_scale,
                    t4T_sb[:], op0=MUL, op1=ADD)

            # ---- phase 2: t5/t8/t9 and final matmul, per column chunk. ------
            # The out-matmuls for chunk j are emitted at j+LAG so the PE never
            # stalls on the serial t4-chain that produces t7 (s==0 case).
            LAG = 2
            o_stage = out_pool.tile([128, 2 * HW], F16, tag="ostage", bufs=2)
            t9_all = {}
            for j in range(NCH2):
                pbase = XM + j * CH_SP
                t9_sb = []
                for g in range(2):
                    # t8: 5 fp8 DoubleRow matmuls (2 taps each) into PSUM
                    t8_ps = t8_psp.tile([128, CH_SP], F32, tag="t8")
                    w8g = w8q_sb[:].rearrange("p (g i a f) -> p g i a f",
                                              g=2, i=5, a=2)
                    for pi, (tapA, tapB) in enumerate(TAP_PAIRS):
                        shA = _sh(tapA)
                        # solo tap: dummy plane B (zero weights) must stay
                        # in-bounds; its shift is +192 so -3 is always safe
                        dlt = (_sh(tapB) - shA) if tapB is not None else -3
                        mov = _mov3(xp8_t[:], g * HWPM + pbase + shA,
                                    dlt, 2, 1, CH_SP)
                        nc.tensor.matmul(
                            t8_ps[:], w8g[:, g, pi], mov,
                            start=(pi == 0), stop=(pi == len(TAP_PAIRS) - 1),
                            perf_mode=DR)
                    # t5: 3 fp8 DoubleRow matmuls computing 16*t5 via the
                    # hl-compensated planes (16*w5h (x)h, w5h (x)l16, w5l (x)h)
                    t5_ps = t5_psp.tile([128, CH_SP], F32, tag="t5")
                    w5g = w5t_sb[:].rearrange("p (g i a f) -> p g i a f",
                                              g=2, i=3, a=2)
                    t5_movs = [(pbase, 2 * HWPM),           # (h0, l0)
                               (HWPM + pbase, 2 * HWPM),    # (h1, l1)
                               (pbase, HWPM)]               # (h0, h1)
                    for mi, (moff, mstride) in enumerate(t5_movs):
                        nc.tensor.matmul(
                            t5_ps[:], w5g[:, g, mi],
                            _mov3(xp8_t[:], moff, mstride, 2, 1, CH_SP),
                            start=(mi == 0), stop=(mi == 2), perf_mode=DR)
                    t5_sb = out_pool.tile([128, CH_OUT], F16, tag="t5sb", bufs=2)
                    nc.scalar.mul(
                        t5_sb[:].rearrange("p (r c) -> p r c", c=56),
                        t5_ps[:].rearrange("p (r c) -> p r c", c=62)[:, :, 3:59],
                        1.0 / 16.0)
                    t9_g = t9_pool.tile([128, CH_OUT], F16, name=f"t9g{g}",
                                        tag="t9c")
                    nc.vector.tensor_max(
                        t9_g[:].rearrange("p (r c) -> p r c", c=56),
                        t5_sb[:].rearrange("p (r c) -> p r c", c=56),
                        t8_ps[:].rearrange("p (r c) -> p r c", c=62)[:, :, 3:59])
                    t9_sb.append(t9_g)
                t9_all[j] = t9_sb
                jos = [j - LAG] if j >= LAG else []
                if j == NCH2 - 1:
                    jos += list(range(NCH2 - LAG, NCH2))
                for jo in jos:
                    _emit_out(nc, small_ps, t7T_sb, t9_all.pop(jo),
                              o_stage, jo)
                    if jo in (3, 5):    # ship completed ranges early
                        lo = 0 if jo == 3 else 4 * CH_OUT
                        hi = (jo + 1) * CH_OUT
                        nc.sync.dma_start(
                            out_dram[s][:, :, lo:hi].rearrange(
                                "t p f -> p t f"),
                            o_stage[:].rearrange("p (t f) -> p t f", t=2)
                            [:, :, lo:hi])
            nc.sync.dma_start(
                out_dram[s][:, :, 6 * CH_OUT:].rearrange("t p f -> p t f"),
                o_stage[:].rearrange("p (t f) -> p t f", t=2)[:, :, 6 * CH_OUT:])


# ---------------------------------------------------------------------------
# host-side input preparation
# ---------------------------------------------------------------------------

F16NP = np.float16
F8NP = ml_dtypes.float8_e4m3fn


def _prep_shared(p1, p2, w4, w5, w8):
    p1 = np.asarray(p1, np.float32)[0]          # [C,H,W]
    p2 = np.asarray(p2, np.float32)[..., 0]     # [H,W,K]
    w4 = np.asarray(w4, np.float32) * (1.0 / np.sqrt(np.float32(C)))
    w5 = np.asarray(w5, np.float32)
    w8 = np.asarray(w8, np.float32)

    p1t = np.zeros((HWPAD, 256), F16NP)
    p1t[:HW] = p1.reshape(C, HW).T
    p2f = np.zeros((HWPAD, 128), F16NP)
    p2f[:HW] = p2.reshape(HW, 128)

    def blockdiag_T(w, kh, kw, dt):
        # out[t][ci, co] = w[t*128+co, ci_local, kh, kw] iff ci//4 == co//4
        out = np.zeros((2, 32, 4, 32, 4), np.float32)
        v = w.reshape(2, 32, 4, 4, 3, 3)        # [t, grp, co_l, ci_l, kh, kw]
        r = np.arange(32)
        out[:, r, :, r, :] = v[:, :, :, :, kh, kw].transpose(1, 0, 3, 2)
        return out.reshape(2, 128, 128).astype(dt)

    w4t = np.stack([blockdiag_T(w4, kh, 1, F16NP) for kh in range(3)])
    # w8 quantized to fp8, packed as DoubleRow tap pairs: [2, 5, 128, 2, 128]
    w8b = {t: blockdiag_T(w8, t[0], t[1], F8NP) for t in TAPS}
    w8q = np.zeros((2, 5, 128, 2, 128), F8NP)
    for g in range(2):
        for pi, (tapA, tapB) in enumerate(TAP_PAIRS):
            w8q[g, pi, :, 0] = w8b[tapA][g]
            if tapB is not None:
                w8q[g, pi, :, 1] = w8b[tapB][g]
    # w5 fp8 hl split; stationaries for 16*t5: (16*w5h, w5h) per ci-half
    # and (w5l_cc0, w5l_cc1); all values in fp8 normal range
    w5h = w5.astype(F8NP).astype(np.float32)
    w5l = ((w5 - w5h) * 16).astype(F8NP).astype(np.float32)
    w5t8 = np.zeros((2, 3, 128, 2, 128), F8NP)
    for dt_ in range(2):
        for cc in range(2):
            hT = w5h[dt_ * 128:(dt_ + 1) * 128,
                     cc * 128:(cc + 1) * 128].T
            lT = w5l[dt_ * 128:(dt_ + 1) * 128,
                     cc * 128:(cc + 1) * 128].T
            w5t8[dt_, cc, :, 0] = (16 * hT).astype(F8NP)
            w5t8[dt_, cc, :, 1] = hT.astype(F8NP)
            w5t8[dt_, 2, :, cc] = lT.astype(F8NP)
    ident = np.eye(128, dtype=F16NP)
    return dict(p1t=p1t.reshape(NCHUNK, 128, 256),
                p2f=p2f.reshape(NCHUNK, 128, 128),
                w4t=w4t, w5t8=w5t8, w8q=w8q, ident=ident)


def _prep_core(x_shard):
    # x_shard: [SPC, C, H, W]
    xs = np.asarray(x_shard, np.float32)
    xpw = np.zeros((SPC, 2, 128, H, PW), np.float32)
    xpw[:, :, :, :, 3:3 + W] = xs.reshape(SPC, 2, 128, H, W)
    xpad = np.zeros((SPC, 2, 128, HWPM), np.float32)
    xpad[:, :, :, XM:XM + HWP] = xpw.reshape(SPC, 2, 128, HWP)
    xt = np.zeros((SPC, HWPAD, 256), F16NP)
    xt[:, :HW] = xs.reshape(SPC, C, HW).transpose(0, 2, 1)
    xpad8 = xpad.astype(F8NP)
    xpad8l = ((xpad - xpad8.astype(np.float32)) * 16).astype(F8NP)
    return dict(xpad8=xpad8, xpad8l=xpad8l,
                xt=xt.reshape(SPC, NCHUNK, 128, 256))


def kernel(x, p1, p2, w4, w5, w8):
    if "nc" not in _PROGRAM_CACHE:
        _PROGRAM_CACHE["nc"] = _build_program()
    nc = _PROGRAM_CACHE["nc"]

    shared = _prep_shared(p1, p2, w4, w5, w8)
    x = np.asarray(x, np.float32)
    in_maps = []
    for c in range(NCORE):
        m = dict(shared)
        m.update(_prep_core(x[c * SPC:(c + 1) * SPC]))
        in_maps.append(m)

    res = run_bass_kernel_spmd(nc, in_maps, core_ids=list(range(NCORE)))
    outs = []
    for c in range(NCORE):
        o = res.results[c]["out"]               # [SPC, 2, 128, HW] fp16
        outs.append(np.asarray(o, np.float32).reshape(SPC, C, H, W))
    return np.concatenate(outs, axis=0)



# revision 4
# speedup vs baseline: 1.1681x; 1.1681x over previous
"""Trainium2 Bass kernel for the dense_cnn problem (min-shipping version).

Math (per sample, C=256, H=W=56, HW=3136, G=2, K=128):
  t1 = p1*x
  t2 = t1[c,hw] @ p2[hw,k]                  (computed transposed: t2T[k,c])
  t3 = t1 @ x.T / sqrt(hw)                  (computed transposed: t3T[d,c])
  t4 = grouped dilated 3x1 conv of t2 (only middle kw tap contributes)
  t5 = w5 @ x
  t8 = grouped dilated 3x3 conv of x (block-diag shifted matmuls)
  t9 = max(t5, t8)
  out = (t4 - t3/sqrt(hw)) @ t9 / sqrt(c)

Precision plan (validated vs reference: rel err ~1.3e-2 < 2e-2 gate):
  - fp16 for x shipping, phase-1 matmuls, t5, t7, t9, out (err ~6e-4)
  - fp8e4m3 for t8 only, via DoubleRow matmuls that pack two taps per
    pass (2x PE rate); t8's error is gated by max(t5,t8).
  - 1/sqrt(c) folded into t7 (w4 and the t3 STT scalar are pre-scaled).

Shipping plan: only x (fp16, natural [c,hw] layout) plus two packed
constant tensors go to the device (~9.6 MB/core vs ~17.7 MB for the
precomputed-everything variant). The hw-major transpose of x is done
on the PE (128x128 identity transposes interleaved with the phase-1
accumulation), and the padded fp8 high/low planes for t5/t8 are
derived on the vector/scalar engines.

Distribution: pure data-parallel over batch, 4 samples per core x 8 cores.
"""

import dataclasses

import numpy as np
import ml_dtypes

import concourse.bass as bass
import concourse.tile as tile
from concourse import bacc, mybir
from concourse.bass_utils import run_bass_kernel_spmd

N, C, H, W = 32, 256, 56, 56
HW = H * W              # 3136
PW = W + 6              # width padded by 3 each side: 62
HWP = H * PW            # 3472
NCORE = 8
SPC = N // NCORE        # samples per core: 4
NCHUNK = 25             # hw-contraction chunks of 128 (rows padded to 3200)
HWPAD = NCHUNK * 128    # 3200
NCH2 = 7                # phase-2 column chunks (8 image rows each)
CH_SP = HWP // NCH2     # 496 padded cols per chunk
CH_OUT = 448            # compact cols per chunk
XM = 192                # zero margin around each padded half (> max |shift| 189)
HWPM = HWP + 2 * XM     # 3856

# packed fp16 constant tensor: p1t | p2f | w4t | ident
O_P1 = 0
O_P2 = NCHUNK * 256             # 6400
O_W4 = O_P2 + NCHUNK * 128      # 9600
O_ID = O_W4 + 6 * 128           # 10368
CW16 = O_ID + 128               # 10496
# packed fp8 constant tensor: w8q | w5t8
O_W8 = 0
O_W5 = 2 * 5 * 2 * 128          # 2560
CW8 = O_W5 + 2 * 3 * 2 * 128    # 4096

F32 = mybir.dt.float32
F16 = mybir.dt.float16
F8 = mybir.dt.float8e4
DR = mybir.MatmulPerfMode.DoubleRow
MUL = mybir.AluOpType.mult
ADD = mybir.AluOpType.add
SUB = mybir.AluOpType.subtract

# t8 tap pairs: (kh,kw) shifts sh = 186*(kh-1) + 3*(kw-1); last pair solo
TAPS = [(kh, kw) for kh in range(3) for kw in range(3)]
TAP_PAIRS = [(TAPS[0], TAPS[1]), (TAPS[2], TAPS[3]), (TAPS[4], TAPS[5]),
             (TAPS[6], TAPS[7]), (TAPS[8], None)]


def _sh(tap):
    kh, kw = tap
    return 186 * (kh - 1) + 3 * (kw - 1)


_PROGRAM_CACHE: dict = {}


def _build_program():
    nc = bacc.Bacc("TRN2", target_bir_lowering=False, debug=False,
                   num_devices=NCORE)

    d = {}
    d["xc"] = nc.dram_tensor("xc", [SPC, 2, 128, HW], F16,
                             kind="ExternalInput").ap()
    d["cw16"] = nc.dram_tensor("cw16", [128, CW16], F16,
                               kind="ExternalInput").ap()
    d["cw8"] = nc.dram_tensor("cw8", [128, CW8], F8,
                              kind="ExternalInput").ap()
    out_dram = nc.dram_tensor("out", [SPC, 2, 128, HW], F16,
                              kind="ExternalOutput").ap()

    with tile.TileContext(nc) as tc:
        _emit(tc, nc, d, out_dram)
    nc.compile()
    return nc


def _mov3(ap, off, d1, n1, d2, n2):
    """Strided (possibly overlapping) 3D view [128, n1, n2] of a 2D tile."""
    return dataclasses.replace(
        ap, offset=ap.offset + off, ap=[ap.ap[0], [d1, n1], [d2, n2]])


def _emit_out(nc, small_ps, t7T_sb, t9_sb, o_stage, j):
    """Final out-matmuls and staging copy for column chunk j."""
    for ct in range(2):
        o_ps = small_ps.tile([128, CH_OUT], F32, name="o_ps", tag="smps")
        for g in range(2):
            nc.tensor.matmul(
                o_ps[:], t7T_sb[g][:, ct * 128:(ct + 1) * 128], t9_sb[g][:],
                start=(g == 0), stop=(g == 1))
        nc.scalar.copy(
            o_stage[:, ct * HW + j * CH_OUT:ct * HW + (j + 1) * CH_OUT],
            o_ps[:])


def _emit(tc, nc, d, out_dram):
    from contextlib import ExitStack
    ctx = ExitStack()
    with ctx:
        const = ctx.enter_context(tc.tile_pool(name="const", bufs=1))
        xc_pool = ctx.enter_context(tc.tile_pool(name="xc", bufs=2))
        xt_pool = ctx.enter_context(tc.tile_pool(name="xt", bufs=3))
        t1_pool = ctx.enter_context(tc.tile_pool(name="t1", bufs=4))
        xpad_pool = ctx.enter_context(tc.tile_pool(name="xpad", bufs=2))
        hh_pool = ctx.enter_context(tc.tile_pool(name="hh", bufs=1))
        xp8_pool = ctx.enter_context(tc.tile_pool(name="xp8", bufs=2))
        t9_pool = ctx.enter_context(tc.tile_pool(name="t9", bufs=6))
        sb_small = ctx.enter_context(tc.tile_pool(name="sbs", bufs=2))
        out_pool = ctx.enter_context(tc.tile_pool(name="outp", bufs=3))
        # PSUM budget (8 banks): acc 2 + small(shared transposes/t4/out) 2 +
        # t8 2 + t5 2 = 8
        acc_ps = ctx.enter_context(tc.tile_pool(name="accps", bufs=1, space="PSUM"))
        small_ps = ctx.enter_context(tc.tile_pool(name="smps", bufs=2, space="PSUM"))
        t8_psp = ctx.enter_context(tc.tile_pool(name="t8ps", bufs=2, space="PSUM"))
        t5_psp = ctx.enter_context(tc.tile_pool(name="t5ps", bufs=2, space="PSUM"))

        # ---- load packed constants (2 contiguous DMAs) ---------------------
        cw16_sb = const.tile([128, CW16], F16)
        cw8_sb = const.tile([128, CW8], F8)
        nc.sync.dma_start(cw16_sb[:], d["cw16"])
        nc.sync.dma_start(cw8_sb[:], d["cw8"])
        p1t_sb = cw16_sb[:, O_P1:O_P2]
        p2f_sb = cw16_sb[:, O_P2:O_W4]
        w4t_sb = cw16_sb[:, O_W4:O_ID]
        id_sb = cw16_sb[:, O_ID:CW16]
        w8q_sb = cw8_sb[:, O_W8:O_W5]
        w5t_sb = cw8_sb[:, O_W5:CW8]
        # padded t2 staging ([128, 2 x 134], pad cols stay zero)
        t2p_sb = const.tile([128, 2 * 134], F16)
        nc.gpsimd.memset(t2p_sb[:], 0.0)

        inv56 = float(1.0 / np.float32(np.sqrt(np.float32(HW))))
        inv16 = float(1.0 / np.float32(np.sqrt(np.float32(C))))
        stt_scale = -inv56 * inv16      # t7 = (w4/16-conv) - t3/(56*16)

        for s in range(SPC):
            # ---- sample DMA: x in natural [c, hw] layout, fp16 ------------
            # 64 extra cols so the full-width chunk-24 transpose of the g=1
            # half stays in-bounds; they must be non-NaN (p1t zeros kill the
            # values).
            xc_t = xc_pool.tile([128, 2 * HW + 64], F16)
            nc.gpsimd.memset(xc_t[:, 2 * HW:], 0.0)
            nc.sync.dma_start(
                xc_t[:, :2 * HW].rearrange("p (t f) -> p t f", t=2),
                d["xc"][s].rearrange("t p f -> p t f"))

            # ---- phase 1: hw-contraction accumulations --------------------
            # Each hw-chunk of x is transposed on the PE ([c,hw] -> [hw,c])
            # one chunk ahead of its accumulation matmuls.
            t2T_ps = acc_ps.tile([128, 256], F32, tag="t2T")
            t3T_ps = acc_ps.tile([128, 512], F32, tag="t3T")

            chunk_tiles = {}

            def emit_tr(i):
                xt_t = xt_pool.tile([128, 256], F16, tag="xtc")
                for g in range(2):
                    xps = small_ps.tile([128, 128], F16, tag="smps")
                    nc.tensor.transpose(
                        xps[:], xc_t[:, g * HW + i * 128:g * HW + i * 128 + 128],
                        id_sb)
                    nc.scalar.copy(xt_t[:, g * 128:(g + 1) * 128], xps[:])
                t1_t = t1_pool.tile([128, 256], F16)
                nc.vector.tensor_mul(t1_t[:], xt_t[:],
                                     p1t_sb[:, i * 256:(i + 1) * 256])
                chunk_tiles[i] = (xt_t, t1_t)

            emit_tr(0)
            for i in range(NCHUNK):
                if i + 1 < NCHUNK:
                    emit_tr(i + 1)
                xt_t, t1_t = chunk_tiles.pop(i)
                nc.tensor.matmul(t2T_ps[:], p2f_sb[:, i * 128:(i + 1) * 128],
                                 t1_t[:], start=(i == 0),
                                 stop=(i == NCHUNK - 1))
                # t3T halves share one PSUM bank (one zero region): only the
                # first matmul starts the group, only the last one stops it.
                for g in range(2):
                    nc.tensor.matmul(t3T_ps[:, g * 256:(g + 1) * 256],
                                     xt_t[:, g * 128:(g + 1) * 128],
                                     t1_t[:],
                                     start=(i == 0 and g == 0),
                                     stop=(i == NCHUNK - 1 and g == 1))

            # ---- derive padded fp8 high/low planes for t5/t8 --------------
            # xpad: [h0 | h1] fp16, each 3856 wide (192 margin + 56x62 rows)
            xpad_t = xpad_pool.tile([128, 2 * HWPM], F16)
            nc.gpsimd.memset(xpad_t[:], 0.0)
            for g in range(2):
                nc.vector.tensor_copy(
                    xpad_t[:, g * HWPM + XM:g * HWPM + XM + HWP]
                    .rearrange("p (r c) -> p r c", c=PW)[:, :, 3:3 + W],
                    xc_t[:, g * HW:(g + 1) * HW]
                    .rearrange("p (r c) -> p r c", c=W))
            # xp8: [h0 | h1 | l0 | l1] fp8; l = 16*(x - fp8(x))
            xp8_t = xp8_pool.tile([128, 4 * HWPM], F8)
            nc.scalar.copy(xp8_t[:, :2 * HWPM], xpad_t[:])
            hh_t = hh_pool.tile([128, 2 * HWPM], F16)
            nc.scalar.mul(hh_t[:], xp8_t[:, :2 * HWPM], 16.0)
            nc.vector.scalar_tensor_tensor(
                xp8_t[:, 2 * HWPM:], xpad_t[:], 16.0, hh_t[:],
                op0=MUL, op1=SUB)

            # ---- phase 1b: t4 chain (tiny, fp16) ----------------------------
            t2T_sb = sb_small.tile([128, 256], F16, tag="t2Tsb")
            nc.scalar.copy(t2T_sb[:], t2T_ps[:])
            for t in range(2):
                t2_ps = small_ps.tile([128, 128], F16, tag="smps")
                nc.tensor.transpose(t2_ps[:], t2T_sb[:, t * 128:(t + 1) * 128],
                                    id_sb)
                nc.vector.tensor_copy(t2p_sb[:, t * 134 + 3:t * 134 + 131],
                                      t2_ps[:])
            t4T_sb = sb_small.tile([128, 256], F16, tag="t4Tsb")
            for t in range(2):
                t4_ps = small_ps.tile([128, 128], F32, tag="smps")
                for ki, kh in enumerate(range(3)):
                    nc.tensor.matmul(
                        t4_ps[:], w4t_sb[:, (kh * 2 + t) * 128:(kh * 2 + t + 1) * 128],
                        t2p_sb[:, t * 134 + 3 * kh:t * 134 + 3 * kh + 128],
                        start=(ki == 0), stop=(ki == 2))
                t4_sb = sb_small.tile([128, 128], F16, tag="t4sb")
                nc.scalar.copy(t4_sb[:], t4_ps[:])
                t4T_ps = small_ps.tile([128, 128], F16, tag="smps")
                nc.tensor.transpose(t4T_ps[:], t4_sb[:], id_sb)
                nc.vector.tensor_copy(t4T_sb[:, t * 128:(t + 1) * 128], t4T_ps[:])
            # t7T[g] = t4T/16 - t3T[g]/(56*16)   (w4t pre-scaled by 1/16)
            t7T_sb = [sb_small.tile([128, 256], F16, name=f"t7T{g}", tag=f"t7T{g}")
                      for g in range(2)]
            for g in range(2):
                nc.vector.scalar_tensor_tensor(
                    t7T_sb[g][:], t3T_ps[:, g * 256:(g + 1) * 256], stt_scale,
                    t4T_sb[:], op0=MUL, op1=ADD)

            # ---- phase 2: t5/t8/t9 and final matmul, per column chunk. ------
            # The out-matmuls for chunk j are emitted at j+LAG so the PE never
            # stalls on the serial t4-chain that produces t7 (s==0 case).
            LAG = 2
            o_stage = out_pool.tile([128, 2 * HW], F16, tag="ostage", bufs=2)
            t9_all = {}
            for j in range(NCH2):
                pbase = XM + j * CH_SP
                t9_sb = []
                for g in range(2):
                    # t8: 5 fp8 DoubleRow matmuls (2 taps each) into PSUM
                    t8_ps = t8_psp.tile([128, CH_SP], F32, tag="t8")
                    w8g = w8q_sb.rearrange("p (g i a f) -> p g i a f",
                                           g=2, i=5, a=2)
                    for pi, (tapA, tapB) in enumerate(TAP_PAIRS):
                        shA = _sh(tapA)
                        # solo tap: dummy plane B (zero weights) must stay
                        # in-bounds; its shift is +192 so -3 is always safe
                        dlt = (_sh(tapB) - shA) if tapB is not None else -3
                        mov = _mov3(xp8_t[:], g * HWPM + pbase + shA,
                                    dlt, 2, 1, CH_SP)
                        nc.tensor.matmul(
                            t8_ps[:], w8g[:, g, pi], mov,
                            start=(pi == 0), stop=(pi == len(TAP_PAIRS) - 1),
                            perf_mode=DR)
                    # t5: 3 fp8 DoubleRow matmuls computing 16*t5 via the
                    # hl-compensated planes (16*w5h (x)h, w5h (x)l16, w5l (x)h)
                    t5_ps = t5_psp.tile([128, CH_SP], F32, tag="t5")
                    w5g = w5t_sb.rearrange("p (g i a f) -> p g i a f",
                                           g=2, i=3, a=2)
                    t5_movs = [(pbase, 2 * HWPM),           # (h0, l0)
                               (HWPM + pbase, 2 * HWPM),    # (h1, l1)
                               (pbase, HWPM)]               # (h0, h1)
                    for mi, (moff, mstride) in enumerate(t5_movs):
                        nc.tensor.matmul(
                            t5_ps[:], w5g[:, g, mi],
                            _mov3(xp8_t[:], moff, mstride, 2, 1, CH_SP),
                            start=(mi == 0), stop=(mi == 2), perf_mode=DR)
                    t5_sb = out_pool.tile([128, CH_OUT], F16, tag="t5sb", bufs=2)
                    nc.scalar.mul(
                        t5_sb[:].rearrange("p (r c) -> p r c", c=56),
                        t5_ps[:].rearrange("p (r c) -> p r c", c=62)[:, :, 3:59],
                        1.0 / 16.0)
                    t9_g = t9_pool.tile([128, CH_OUT], F16, name=f"t9g{g}",
                                        tag="t9c")
                    nc.vector.tensor_max(
                        t9_g[:].rearrange("p (r c) -> p r c", c=56),
                        t5_sb[:].rearrange("p (r c) -> p r c", c=56),
                        t8_ps[:].rearrange("p (r c) -> p r c", c=62)[:, :, 3:59])
                    t9_sb.append(t9_g)
                t9_all[j] = t9_sb
                jos = [j - LAG] if j >= LAG else []
                if j == NCH2 - 1:
                    jos += list(range(NCH2 - LAG, NCH2))
                for jo in jos:
                    _emit_out(nc, small_ps, t7T_sb, t9_all.pop(jo),
                              o_stage, jo)
            nc.sync.dma_start(
                out_dram[s].rearrange("t p f -> p t f"),
                o_stage[:].rearrange("p (t f) -> p t f", t=2))


# ---------------------------------------------------------------------------
# host-side input preparation
# ---------------------------------------------------------------------------

F16NP = np.float16
F8NP = ml_dtypes.float8_e4m3fn


def _prep_shared(p1, p2, w4, w5, w8):
    p1 = np.asarray(p1, np.float32)[0]          # [C,H,W]
    p2 = np.asarray(p2, np.float32)[..., 0]     # [H,W,K]
    w4 = np.asarray(w4, np.float32) * (1.0 / np.sqrt(np.float32(C)))
    w5 = np.asarray(w5, np.float32)
    w8 = np.asarray(w8, np.float32)

    p1t = np.zeros((HWPAD, 256), F16NP)
    p1t[:HW] = p1.reshape(C, HW).T
    p2f = np.zeros((HWPAD, 128), F16NP)
    p2f[:HW] = p2.reshape(HW, 128)

    def blockdiag_T(w, kh, kw, dt):
        # out[t][ci, co] = w[t*128+co, ci_local, kh, kw] iff ci//4 == co//4
        out = np.zeros((2, 32, 4, 32, 4), np.float32)
        v = w.reshape(2, 32, 4, 4, 3, 3)        # [t, grp, co_l, ci_l, kh, kw]
        r = np.arange(32)
        out[:, r, :, r, :] = v[:, :, :, :, kh, kw].transpose(1, 0, 3, 2)
        return out.reshape(2, 128, 128).astype(dt)

    w4t = np.stack([blockdiag_T(w4, kh, 1, F16NP) for kh in range(3)])
    # w8 quantized to fp8, packed as DoubleRow tap pairs: [2, 5, 128, 2, 128]
    w8b = {t: blockdiag_T(w8, t[0], t[1], F8NP) for t in TAPS}
    w8q = np.zeros((2, 5, 128, 2, 128), F8NP)
    for g in range(2):
        for pi, (tapA, tapB) in enumerate(TAP_PAIRS):
            w8q[g, pi, :, 0] = w8b[tapA][g]
            if tapB is not None:
                w8q[g, pi, :, 1] = w8b[tapB][g]
    # w5 fp8 hl split; stationaries for 16*t5: (16*w5h, w5h) per ci-half
    # and (w5l_cc0, w5l_cc1); all values in fp8 normal range
    w5h = w5.astype(F8NP).astype(np.float32)
    w5l = ((w5 - w5h) * 16).astype(F8NP).astype(np.float32)
    w5t8 = np.zeros((2, 3, 128, 2, 128), F8NP)
    for dt_ in range(2):
        for cc in range(2):
            hT = w5h[dt_ * 128:(dt_ + 1) * 128,
                     cc * 128:(cc + 1) * 128].T
            lT = w5l[dt_ * 128:(dt_ + 1) * 128,
                     cc * 128:(cc + 1) * 128].T
            w5t8[dt_, cc, :, 0] = (16 * hT).astype(F8NP)
            w5t8[dt_, cc, :, 1] = hT.astype(F8NP)
            w5t8[dt_, 2, :, cc] = lT.astype(F8NP)

    cw16 = np.zeros((128, CW16), F16NP)
    cw16[:, O_P1:O_P2] = (p1t.reshape(NCHUNK, 128, 256)
                          .transpose(1, 0, 2).reshape(128, NCHUNK * 256))
    cw16[:, O_P2:O_W4] = (p2f.reshape(NCHUNK, 128, 128)
                          .transpose(1, 0, 2).reshape(128, NCHUNK * 128))
    cw16[:, O_W4:O_ID] = (w4t.transpose(2, 0, 1, 3)   # [128, 3, 2, 128]
                          .reshape(128, 6 * 128))
    cw16[:, O_ID:CW16] = np.eye(128, dtype=F16NP)
    cw8 = np.zeros((128, CW8), F8NP)
    cw8[:, O_W8:O_W5] = (w8q.transpose(2, 0, 1, 3, 4)
                         .reshape(128, 2 * 5 * 2 * 128))
    cw8[:, O_W5:CW8] = (w5t8.transpose(2, 0, 1, 3, 4)
                        .reshape(128, 2 * 3 * 2 * 128))
    return dict(cw16=cw16, cw8=cw8)


def _prep_core(x_shard):
    # x_shard: [SPC, C, H, W] -> natural [c, hw] fp16 layout
    xs = np.asarray(x_shard)
    return dict(xc=np.ascontiguousarray(
        xs.reshape(SPC, 2, 128, HW).astype(F16NP)))


def kernel(x, p1, p2, w4, w5, w8):
    if "nc" not in _PROGRAM_CACHE:
        _PROGRAM_CACHE["nc"] = _build_program()
    nc = _PROGRAM_CACHE["nc"]

    shared = _prep_shared(p1, p2, w4, w5, w8)
    x = np.asarray(x, np.float32)
    in_maps = []
    for c in range(NCORE):
        m = dict(shared)
        m.update(_prep_core(x[c * SPC:(c + 1) * SPC]))
        in_maps.append(m)

    res = run_bass_kernel_spmd(nc, in_maps, core_ids=list(range(NCORE)))
    outs = []
    for c in range(NCORE):
        o = res.results[c]["out"]               # [SPC, 2, 128, HW] fp16
        outs.append(np.asarray(o, np.float32).reshape(SPC, C, H, W))
    return np.concatenate(outs, axis=0)


# revision 35
# speedup vs baseline: 1.1985x; 1.0261x over previous
"""Trainium2 Bass kernel for the dense_cnn problem (min-shipping version).

Math (per sample, C=256, H=W=56, HW=3136, G=2, K=128):
  t1 = p1*x
  t2 = t1[c,hw] @ p2[hw,k]                  (computed transposed: t2T[k,c])
  t3 = t1 @ x.T / sqrt(hw)                  (computed transposed: t3T[d,c])
  t4 = grouped dilated 3x1 conv of t2 (only middle kw tap contributes)
  t5 = w5 @ x
  t8 = grouped dilated 3x3 conv of x (block-diag shifted matmuls)
  t9 = max(t5, t8)
  out = (t4 - t3/sqrt(hw)) @ t9 / sqrt(c)

Precision plan (validated vs reference: rel err ~1.39e-2 < 2e-2 gate):
  - fp16 for x shipping, phase-1 matmuls, t5, t7, t9, out
  - fp8e4m3 for t8 only, via DoubleRow matmuls that pack two taps per
    pass (2x PE rate); t8's error is gated by max(t5,t8).
  - 1/sqrt(c) folded into t7 (w4 and the t3 STT scalar are pre-scaled).

Shipping plan: only x (fp16, natural [c,hw] layout) plus two packed
constant tensors go to the device (~9.6 MB/core vs ~17.7 MB for the
precomputed-everything variant; 76.5 MB total vs 141 MB). The hw-major
[hw, c] copy of x that phase 1 contracts over is produced on-device by
two XBAR transpose-DMAs per sample (out[p, i, :] = in[:, i*128+p]^T
matches the chunk-major layout exactly), and the zero-padded fp16/fp8
planes for t5/t8 are derived on the vector/gpsimd engines. Device exec
(TimelineSim model): ~138 us vs ~129 us for the precomputed variant.

HW constraints hit while building this (CoreSim does not model them):
  - an engine instruction may read at most ONE operand from PSUM
  - GPSIMD cannot access PSUM at all
  - matmul/PSUM output dtype must be fp32 on TRN2

Distribution: pure data-parallel over batch, 4 samples per core x 8 cores.
"""

import dataclasses

import numpy as np
import ml_dtypes

import concourse.bass as bass
import concourse.tile as tile
from concourse import bacc, mybir
from concourse.bass_utils import run_bass_kernel_spmd

N, C, H, W = 32, 256, 56, 56
HW = H * W              # 3136
PW = W + 6              # width padded by 3 each side: 62
HWP = H * PW            # 3472
NCORE = 8
SPC = N // NCORE        # samples per core: 4
NCHUNK = 25             # hw-contraction chunks of 128 (rows padded to 3200)
HWPAD = NCHUNK * 128    # 3200
NCH2 = 7                # phase-2 column chunks (8 image rows each)
CH_SP = HWP // NCH2     # 496 padded cols per chunk
CH_OUT = 448            # compact cols per chunk
XM = 192                # zero margin around each padded half (> max |shift| 189)
HWPM = HWP + 2 * XM     # 3856

# packed fp16 constant tensor: p1t | p2f | w4t | w5t | ident
O_P1 = 0
O_P2 = NCHUNK * 256             # 6400
O_W4 = O_P2 + NCHUNK * 128      # 9600
O_W5 = O_W4 + 6 * 128           # 10368
O_ID = O_W5 + 4 * 128           # 10880
CW16 = O_ID + 128               # 11008
# packed fp8 constant tensor: w8q
CW8 = 2 * 5 * 2 * 128           # 2560

F32 = mybir.dt.float32
F16 = mybir.dt.float16
F8 = mybir.dt.float8e4
DR = mybir.MatmulPerfMode.DoubleRow
MUL = mybir.AluOpType.mult
ADD = mybir.AluOpType.add
SUB = mybir.AluOpType.subtract

# t8 tap pairs: (kh,kw) shifts sh = 186*(kh-1) + 3*(kw-1); last pair solo
TAPS = [(kh, kw) for kh in range(3) for kw in range(3)]
TAP_PAIRS = [(TAPS[0], TAPS[1]), (TAPS[2], TAPS[3]), (TAPS[4], TAPS[5]),
             (TAPS[6], TAPS[7]), (TAPS[8], None)]


def _sh(tap):
    kh, kw = tap
    return 186 * (kh - 1) + 3 * (kw - 1)


_PROGRAM_CACHE: dict = {}


def _build_program():
    nc = bacc.Bacc("TRN2", target_bir_lowering=False, debug=False,
                   num_devices=NCORE)

    d = {}
    d["xc"] = nc.dram_tensor("xc", [SPC, 2, 128, HWPAD], F16,
                             kind="ExternalInput").ap()
    d["cw16"] = nc.dram_tensor("cw16", [128, CW16], F16,
                               kind="ExternalInput").ap()
    d["cw8"] = nc.dram_tensor("cw8", [128, CW8], F8,
                              kind="ExternalInput").ap()
    out_dram = nc.dram_tensor("out", [SPC, 2, 128, HW], F16,
                              kind="ExternalOutput").ap()

    with tile.TileContext(nc) as tc:
        _emit(tc, nc, d, out_dram)
    nc.compile()
    return nc


def _mov3(ap, off, d1, n1, d2, n2):
    """Strided (possibly overlapping) 3D view [128, n1, n2] of a 2D tile."""
    return dataclasses.replace(
        ap, offset=ap.offset + off, ap=[ap.ap[0], [d1, n1], [d2, n2]])


def _emit_out(nc, small_ps, t7T_sb, t9_sb, o_stage, j):
    """Final out-matmuls and staging copy for column chunk j."""
    for ct in range(2):
        o_ps = small_ps.tile([128, CH_OUT], F32, name="o_ps", tag="smps")
        for g in range(2):
            nc.tensor.matmul(
                o_ps[:], t7T_sb[g][:, ct * 128:(ct + 1) * 128], t9_sb[g][:],
                start=(g == 0), stop=(g == 1))
        nc.scalar.copy(
            o_stage[:, ct * HW + j * CH_OUT:ct * HW + (j + 1) * CH_OUT],
            o_ps[:])


def _emit(tc, nc, d, out_dram):
    from contextlib import ExitStack
    ctx = ExitStack()
    with ctx:
        const = ctx.enter_context(tc.tile_pool(name="const", bufs=1))
        xc_pool = ctx.enter_context(tc.tile_pool(name="xc", bufs=3))
        xt_pool = ctx.enter_context(tc.tile_pool(name="xt", bufs=3))
        t1_pool = ctx.enter_context(tc.tile_pool(name="t1", bufs=4))
        xp8_pool = ctx.enter_context(tc.tile_pool(name="xp8", bufs=2))
        t9_pool = ctx.enter_context(tc.tile_pool(name="t9", bufs=6))
        sb_small = ctx.enter_context(tc.tile_pool(name="sbs", bufs=2))
        out_pool = ctx.enter_context(tc.tile_pool(name="outp", bufs=3))
        # PSUM budget (8 banks): acc 2 + small(shared transposes/t4/out) 2 +
        # t8 2 + t5 2 = 8
        acc_ps = ctx.enter_context(tc.tile_pool(name="accps", bufs=1, space="PSUM"))
        small_ps = ctx.enter_context(tc.tile_pool(name="smps", bufs=2, space="PSUM"))
        t8_psp = ctx.enter_context(tc.tile_pool(name="t8ps", bufs=2, space="PSUM"))
        t5_psp = ctx.enter_context(tc.tile_pool(name="t5ps", bufs=2, space="PSUM"))

        # ---- load packed constants -----------------------------------------
        # p1t/p2f stream in blocks behind sample 0's xc load + transposes so
        # phase 1 can start as soon as the first chunks land; the small tail
        # (w4/w5/ident + w8) rides the Activation queue.
        cw16_sb = const.tile([128, CW16], F16)
        cw8_sb = const.tile([128, CW8], F8)
        nc.scalar.dma_start(cw16_sb[:, O_W4:], d["cw16"][:, O_W4:])
        nc.scalar.dma_start(cw8_sb[:], d["cw8"])
        p1t_sb = cw16_sb[:, O_P1:O_P2]
        p2f_sb = cw16_sb[:, O_P2:O_W4]
        w4t_sb = cw16_sb[:, O_W4:O_W5]
        w5t_sb = cw16_sb[:, O_W5:O_ID]
        id_sb = cw16_sb[:, O_ID:CW16]
        w8q_sb = cw8_sb[:]
        # padded t2 staging ([128, 2 x 134], pad cols stay zero)
        t2p_sb = const.tile([128, 2 * 134], F16)
        nc.gpsimd.memset(t2p_sb[:], 0.0)

        inv56 = float(1.0 / np.float32(np.sqrt(np.float32(HW))))
        inv16 = float(1.0 / np.float32(np.sqrt(np.float32(C))))
        stt_scale = -inv56 * inv16      # t7 = (w4/16-conv) - t3/(56*16)

        for s in range(SPC):
            # ---- sample DMA: x in natural [c, hw] layout, fp16, padded ----
            # (each half padded to 3200 cols of zeros host-side so the
            # DRAM-direct XBAR transpose covers all 25 chunks exactly)
            xc_t = xc_pool.tile([128, 2 * HWPAD], F16)
            nc.sync.dma_start(
                xc_t[:].rearrange("p (t f) -> p t f", t=2),
                d["xc"][s].rearrange("t p f -> p t f"))

            # ---- hw-major x via two XBAR transpose-DMAs -------------------
            # out[p, i, :] = xc[:, i*128+p]^T, i.e. the chunk-major [hw, c]
            # layout phase 1 wants. The 64-col zero tail lets g=0 read 3200
            # cols (the 64 overhang rows hit zero p1t/p2f rows).
            xt_t = xt_pool.tile([128, NCHUNK * 256], F16)
            xtv = xt_t[:].rearrange("p (i f) -> p i f", f=256)
            for g in range(2):
                nc.sync.dma_start_transpose(
                    xtv[:, :, g * 128:(g + 1) * 128],
                    xc_t[:, g * HWPAD:(g + 1) * HWPAD])
            if s == 0:
                # p1t/p2f in 5-chunk blocks, behind the critical s0 DMAs
                for b0 in range(0, NCHUNK, 5):
                    nc.sync.dma_start(
                        cw16_sb[:, b0 * 256:(b0 + 5) * 256],
                        d["cw16"][:, b0 * 256:(b0 + 5) * 256])
                    nc.sync.dma_start(
                        cw16_sb[:, O_P2 + b0 * 128:O_P2 + (b0 + 5) * 128],
                        d["cw16"][:, O_P2 + b0 * 128:O_P2 + (b0 + 5) * 128])

            # ---- padded fp8 planes for t8 (margins/row-pads zeroed) -------
            # xp8: [x0 | x1] fp8, each 3856 wide (192 margin + 56x62 rows);
            # filled below (after phase 1) by strided f16->f8 casts of xc
            xp8_t = xp8_pool.tile([128, 2 * HWPM], F8)
            for g in range(2):
                base = g * HWPM
                nc.gpsimd.memset(xp8_t[:, base:base + XM + 3], 0.0)
                nc.gpsimd.memset(
                    xp8_t[:, base + XM + 59:base + XM + 59 + 55 * PW]
                    .rearrange("p (r c) -> p r c", c=PW)[:, :, 0:6], 0.0)
                nc.gpsimd.memset(
                    xp8_t[:, base + HWPM - XM - 3:base + HWPM], 0.0)
                # data fill: strided f16->f8 cast of xc, on the otherwise
                # idle gpsimd queue (runs during the previous sample's
                # phase 2 / this sample's phase 1)
                nc.gpsimd.tensor_copy(
                    xp8_t[:, base + XM:base + XM + HWP]
                    .rearrange("p (r c) -> p r c", c=PW)[:, :, 3:3 + W],
                    xc_t[:, g * HWPAD:g * HWPAD + HW]
                    .rearrange("p (r c) -> p r c", c=W))

            # ---- phase 1: hw-contraction accumulations --------------------
            t2T_ps = acc_ps.tile([128, 256], F32, tag="t2T")
            t3T_ps = acc_ps.tile([128, 512], F32, tag="t3T")
            TB = 5              # t1 mul batch: 5 chunks per DVE op
            t1_tiles = {}
            for i in range(NCHUNK):
                if i % TB == 0:
                    t1_t = t1_pool.tile([128, TB * 256], F16)
                    nc.vector.tensor_mul(
                        t1_t[:], xt_t[:, i * 256:(i + TB) * 256],
                        p1t_sb[:, i * 256:(i + TB) * 256])
                    t1_tiles[i // TB] = t1_t
                t1c = t1_tiles[i // TB][:, (i % TB) * 256:(i % TB + 1) * 256]
                nc.tensor.matmul(t2T_ps[:], p2f_sb[:, i * 128:(i + 1) * 128],
                                 t1c, start=(i == 0),
                                 stop=(i == NCHUNK - 1))
                # t3T halves share one PSUM bank (one zero region): only the
                # first matmul starts the group, only the last one stops it.
                for g in range(2):
                    nc.tensor.matmul(t3T_ps[:, g * 256:(g + 1) * 256],
                                     xt_t[:, i * 256 + g * 128:
                                          i * 256 + (g + 1) * 128],
                                     t1c,
                                     start=(i == 0 and g == 0),
                                     stop=(i == NCHUNK - 1 and g == 1))

            # ---- phase 1b: t4 chain (tiny, fp16; transposes via XBAR) -------
            t2T_sb = sb_small.tile([128, 256], F16, tag="t2Tsb")
            nc.scalar.copy(t2T_sb[:], t2T_ps[:])
            for t in range(2):
                t2_ps = small_ps.tile([128, 128], F16, tag="smps")
                nc.tensor.transpose(t2_ps[:], t2T_sb[:, t * 128:(t + 1) * 128],
                                    id_sb)
                nc.vector.tensor_copy(t2p_sb[:, t * 134 + 3:t * 134 + 131],
                                      t2_ps[:])
            t4T_sb = sb_small.tile([128, 256], F16, tag="t4Tsb")
            for t in range(2):
                t4_ps = small_ps.tile([128, 128], F32, tag="smps")
                for ki, kh in enumerate(range(3)):
                    nc.tensor.matmul(
                        t4_ps[:], w4t_sb[:, (kh * 2 + t) * 128:(kh * 2 + t + 1) * 128],
                        t2p_sb[:, t * 134 + 3 * kh:t * 134 + 3 * kh + 128],
                        start=(ki == 0), stop=(ki == 2))
                t4_sb = sb_small.tile([128, 128], F16, tag="t4sb")
                nc.scalar.copy(t4_sb[:], t4_ps[:])
                t4T_ps = small_ps.tile([128, 128], F16, tag="smps")
                nc.tensor.transpose(t4T_ps[:], t4_sb[:], id_sb)
                nc.vector.tensor_copy(t4T_sb[:, t * 128:(t + 1) * 128],
                                      t4T_ps[:])
            # t7T[g] = t4T/16 - t3T[g]/(56*16)   (w4t pre-scaled by 1/16)
            t7T_sb = [sb_small.tile([128, 256], F16, name=f"t7T{g}", tag=f"t7T{g}")
                      for g in range(2)]
            for g in range(2):
                nc.vector.scalar_tensor_tensor(
                    t7T_sb[g][:], t3T_ps[:, g * 256:(g + 1) * 256], stt_scale,
                    t4T_sb[:], op0=MUL, op1=ADD)

            # ---- phase 2: t5/t8/t9 and final matmul, per column chunk. ------
            # The out-matmuls for chunk j are emitted at j+LAG so the PE never
            # stalls on the serial t4-chain that produces t7 (s==0 case).
            LAG = 2
            o_stage = out_pool.tile([128, 2 * HW], F16, tag="ostage", bufs=2)
            t9_all = {}
            for j in range(NCH2):
                pbase = XM + j * CH_SP
                t9_sb = []
                for g in range(2):
                    # t8: 5 fp8 DoubleRow matmuls (2 taps each) into PSUM
                    t8_ps = t8_psp.tile([128, CH_SP], F32, tag="t8")
                    w8g = w8q_sb.rearrange("p (g i a f) -> p g i a f",
                                           g=2, i=5, a=2)
                    for pi, (tapA, tapB) in enumerate(TAP_PAIRS):
                        shA = _sh(tapA)
                        # solo tap: dummy plane B (zero weights) must stay
                        # in-bounds; its shift is +192 so -3 is always safe
                        dlt = (_sh(tapB) - shA) if tapB is not None else -3
                        mov = _mov3(xp8_t[:], g * HWPM + pbase + shA,
                                    dlt, 2, 1, CH_SP)
                        nc.tensor.matmul(
                            t8_ps[:], w8g[:, g, pi], mov,
                            start=(pi == 0), stop=(pi == len(TAP_PAIRS) - 1),
                            perf_mode=DR)
                    # t5 has no spatial shifts: 2 fp16 matmuls directly on
                    # the compact [c, hw] tile (448 cols, no pad overhead)
                    t5_ps = t5_psp.tile([128, CH_OUT], F32, tag="t5")
                    for ci in range(2):
                        nc.tensor.matmul(
                            t5_ps[:],
                            w5t_sb[:, (ci * 2 + g) * 128:(ci * 2 + g + 1) * 128],
                            xc_t[:, ci * HWPAD + j * CH_OUT:
                                 ci * HWPAD + (j + 1) * CH_OUT],
                            start=(ci == 0), stop=(ci == 1))
                    # engines can read only ONE operand from PSUM: stage t5
                    # through SBUF (ACT), then max against t8's PSUM tile
                    t5_sb = out_pool.tile([128, CH_OUT], F16, tag="t5sb", bufs=2)
                    nc.scalar.copy(t5_sb[:], t5_ps[:])
                    t9_g = t9_pool.tile([128, CH_OUT], F16, name=f"t9g{g}",
                                        tag="t9c")
                    nc.vector.tensor_max(
                        t9_g[:].rearrange("p (r c) -> p r c", c=56),
                        t5_sb[:].rearrange("p (r c) -> p r c", c=56),
                        t8_ps[:].rearrange("p (r c) -> p r c", c=62)[:, :, 3:59])
                    t9_sb.append(t9_g)
                t9_all[j] = t9_sb
                jos = [j - LAG] if j >= LAG else []
                if j == NCH2 - 1:
                    jos += list(range(NCH2 - LAG, NCH2))
                for jo in jos:
                    _emit_out(nc, small_ps, t7T_sb, t9_all.pop(jo),
                              o_stage, jo)
            nc.scalar.dma_start(
                out_dram[s].rearrange("t p f -> p t f"),
                o_stage[:].rearrange("p (t f) -> p t f", t=2))


# ---------------------------------------------------------------------------
# host-side input preparation
# ---------------------------------------------------------------------------

F16NP = np.float16
F8NP = ml_dtypes.float8_e4m3fn


def _prep_shared(p1, p2, w4, w5, w8):
    p1 = np.asarray(p1, np.float32)[0]          # [C,H,W]
    p2 = np.asarray(p2, np.float32)[..., 0]     # [H,W,K]
    w4 = np.asarray(w4, np.float32) * (1.0 / np.sqrt(np.float32(C)))
    w5 = np.asarray(w5, np.float32)
    w8 = np.asarray(w8, np.float32)

    p1t = np.zeros((HWPAD, 256), F16NP)
    p1t[:HW] = p1.reshape(C, HW).T
    p2f = np.zeros((HWPAD, 128), F16NP)
    p2f[:HW] = p2.reshape(HW, 128)

    def blockdiag_T(w, kh, kw, dt):
        # out[t][ci, co] = w[t*128+co, ci_local, kh, kw] iff ci//4 == co//4
        out = np.zeros((2, 32, 4, 32, 4), np.float32)
        v = w.reshape(2, 32, 4, 4, 3, 3)        # [t, grp, co_l, ci_l, kh, kw]
        r = np.arange(32)
        out[:, r, :, r, :] = v[:, :, :, :, kh, kw].transpose(1, 0, 3, 2)
        return out.reshape(2, 128, 128).astype(dt)

    w4t = np.stack([blockdiag_T(w4, kh, 1, F16NP) for kh in range(3)])
    # w8 quantized to fp8, packed as DoubleRow tap pairs: [2, 5, 128, 2, 128]
    w8b = {t: blockdiag_T(w8, t[0], t[1], F8NP) for t in TAPS}
    w8q = np.zeros((2, 5, 128, 2, 128), F8NP)
    for g in range(2):
        for pi, (tapA, tapB) in enumerate(TAP_PAIRS):
            w8q[g, pi, :, 0] = w8b[tapA][g]
            if tapB is not None:
                w8q[g, pi, :, 1] = w8b[tapB][g]
    # w5 fp16 stationaries: block (ci, co) = w5[co-half, ci-half].T
    w5t = np.zeros((128, 4 * 128), F16NP)
    for ci in range(2):
        for co in range(2):
            w5t[:, (ci * 2 + co) * 128:(ci * 2 + co + 1) * 128] = (
                w5[co * 128:(co + 1) * 128, ci * 128:(ci + 1) * 128].T)

    cw16 = np.zeros((128, CW16), F16NP)
    cw16[:, O_P1:O_P2] = (p1t.reshape(NCHUNK, 128, 256)
                          .transpose(1, 0, 2).reshape(128, NCHUNK * 256))
    cw16[:, O_P2:O_W4] = (p2f.reshape(NCHUNK, 128, 128)
                          .transpose(1, 0, 2).reshape(128, NCHUNK * 128))
    cw16[:, O_W4:O_W5] = (w4t.transpose(2, 0, 1, 3)   # [128, 3, 2, 128]
                          .reshape(128, 6 * 128))
    cw16[:, O_W5:O_ID] = w5t
    cw16[:, O_ID:CW16] = np.eye(128, dtype=F16NP)
    cw8 = np.ascontiguousarray(
        w8q.transpose(2, 0, 1, 3, 4).reshape(128, 2 * 5 * 2 * 128))
    return dict(cw16=cw16, cw8=cw8)


def _prep_core(x_shard):
    # x_shard: [SPC, C, H, W] -> natural [c, hw] fp16 layout, hw padded
    # with zeros to 3200 so the XBAR transpose covers 25 full chunks
    xs = np.asarray(x_shard)
    xc = np.zeros((SPC, 2, 128, HWPAD), F16NP)
    xc[..., :HW] = xs.reshape(SPC, 2, 128, HW)
    return dict(xc=xc)


def kernel(x, p1, p2, w4, w5, w8):
    if "nc" not in _PROGRAM_CACHE:
        _PROGRAM_CACHE["nc"] = _build_program()
    nc = _PROGRAM_CACHE["nc"]

    shared = _prep_shared(p1, p2, w4, w5, w8)
    x = np.asarray(x, np.float32)
    in_maps = []
    for c in range(NCORE):
        m = dict(shared)
        m.update(_prep_core(x[c * SPC:(c + 1) * SPC]))
        in_maps.append(m)

    res = run_bass_kernel_spmd(nc, in_maps, core_ids=list(range(NCORE)))
    outs = []
    for c in range(NCORE):
        o = res.results[c]["out"]               # [SPC, 2, 128, HW] fp16
        outs.append(np.asarray(o, np.float32).reshape(SPC, C, H, W))
    return np.concatenate(outs, axis=0)
